# revision 24
# baseline (speedup 1.0000x reference)
"""AttnBlock3D Trainium2 Bass kernel (8 NeuronCores, SPMD), v3.

Per core r: heads n = 2r, 2r+1.  x viewed as [128=(t,c), 4096=hw].

Structure:
  * ONE persistent [128, 3072] f32 PSUM tile = 6 rotating 512-col slots
    (bank-aligned).  Subtile dependency tracking gives the pipeline:
    QK matmul m writes slot m%6 (waits the exp that read it 6 slots
    ago); exp ops read slot PAIRS [128,1024] (mixed heads - exp is
    elementwise); AV consumes ex halves via a slot map.
  * QK matmuls are emitted in pure round-robin over the 4 PE row
    groups (head l replicated at rows 32l and 32l+64, with the bq bias
    folded into k row 8 and q row 8 = ones), so 4 run concurrently.
    AV matmuls (K=128, all rows) for group g-1 are emitted after the
    QK matmuls of group g so they never fence the QK streams.
  * exp split: ACT (exact, bias -c*ln2/128) and DVE (Schraudolph
    int16 bit-trick, bitcast to bf16), alternating slot pairs.
  * i processed in 4 quarters; AV accumulates both heads in one psum
    bank (col group 2l+b); per-quarter bf16 AllGather + output
    projection overlap the following quarters.
"""
import math
import sys

import numpy as np

sys.path.insert(0, "/opt/trn_rl_repo")

T, C, HW, NCORES = 8, 16, 4096, 8
N_ELEM = T * HW
EPS = 1e-5
SCALE = float(T) ** -0.5
NQ = 4            # i-quarters
QW = HW // NQ     # 1024
# Schraudolph: i16 = A*s + B; bf16-bits(i16) ~= exp(SCALE*s - CBIAS)
A_SCH = SCALE * 128.0 / math.log(2.0)
C_SCH = 7.422
B_SCH = 16256.0 - C_SCH
CBIAS = C_SCH * math.log(2.0) / 128.0

_CACHE = {}


def _build_program():
    import concourse.bass as bass
    import concourse.bacc as bacc
    import concourse.tile as tile
    from concourse import mybir

    f32 = mybir.dt.float32
    bf16 = mybir.dt.bfloat16
    i16 = mybir.dt.int16
    AX = mybir.AxisListType
    OP = mybir.AluOpType
    ACT = mybir.ActivationFunctionType

    nc = bacc.Bacc("TRN2", target_bir_lowering=False, debug=False,
                   num_devices=NCORES)
    x = nc.dram_tensor("x", [128, HW], f32, kind="ExternalInput").ap()
    wq_bd = nc.dram_tensor("wq_bd", [128, 128], bf16,
                           kind="ExternalInput").ap()
    wk_bd = nc.dram_tensor("wk_bd", [128, 128], bf16,
                           kind="ExternalInput").ap()
    wv_rhs = nc.dram_tensor("wv_rhs", [128, 18], bf16,
                            kind="ExternalInput").ap()
    wp_bd = nc.dram_tensor("wp_bd", [128, 128], bf16,
                           kind="ExternalInput").ap()
    bp_col = nc.dram_tensor("bp_col", [128, 1], f32, kind="ExternalInput").ap()
    sel = nc.dram_tensor("sel", [128, 16], f32, kind="ExternalInput").ap()
    ones_row = nc.dram_tensor("ones_row", [1, HW], bf16,
                              kind="ExternalInput").ap()
    out = nc.dram_tensor("out", [128, HW], f32, kind="ExternalOutput").ap()

    cc_in = [nc.dram_tensor(f"cc_in{q}", [18, QW], bf16).ap()
             for q in range(NQ)]
    cc_out = [nc.dram_tensor(f"cc_out{q}", [NCORES * 18, QW], bf16,
                             addr_space="Shared").ap() for q in range(NQ)]

    with tile.TileContext(nc) as tc:
        with (
            tc.tile_pool(name="persist", bufs=1) as P1,
            tc.tile_pool(name="work", bufs=4) as PW,
            tc.tile_pool(name="ex", bufs=4) as PE_,
            tc.tile_pool(name="scratch", bufs=1) as PS,
            tc.tile_pool(name="psum", bufs=1, space="PSUM") as PSQ,
            tc.tile_pool(name="dram", bufs=2, space="DRAM") as PD,
        ):
            # psum budget (8 banks): qk 3x[128,1024]=6, av 1, pp 1
            # ---------------- loads ----------------
            x_sb = P1.tile([128, HW], f32)
            nc.sync.dma_start(out=x_sb[:, 0:2048], in_=x[:, 0:2048])
            nc.sync.dma_start(out=x_sb[:, 2048:4096], in_=x[:, 2048:4096])
            wqbd_sb = P1.tile([128, 128], bf16)
            nc.sync.dma_start(out=wqbd_sb, in_=wq_bd)
            wkbd_sb = P1.tile([128, 128], bf16)
            nc.sync.dma_start(out=wkbd_sb, in_=wk_bd)
            wvrhs_sb = P1.tile([128, 18], bf16)
            nc.sync.dma_start(out=wvrhs_sb, in_=wv_rhs)
            wpbd_sb = P1.tile([128, 128], bf16)
            nc.sync.dma_start(out=wpbd_sb, in_=wp_bd)
            bpcol_sb = P1.tile([128, 1], f32)
            nc.sync.dma_start(out=bpcol_sb, in_=bp_col)
            sel_sb = P1.tile([128, 16], f32)
            nc.sync.dma_start(out=sel_sb, in_=sel)

            # PE warm-up: ~3.5us of back-to-back matmuls during the BN
            # stats phase so the HAM clock gate lifts to 2.4 GHz.
            wu = PSQ.tile([128, 512], f32, tag="pp")
            for w in range(32):
                nc.tensor.matmul(wu[:, 0:128], lhsT=wqbd_sb,
                                 rhs=wkbd_sb, start=True, stop=True)

            # ---------------- BN stats ----------------
            s1 = P1.tile([128, 4], f32)
            sq_scr = PS.tile([128, 2048], bf16, tag="sqscr")
            for h in range(2):
                cs = slice(h * 2048, (h + 1) * 2048)
                nc.vector.reduce_sum(out=s1[:, h:h + 1], in_=x_sb[:, cs],
                                     axis=AX.X)
                nc.scalar.activation(sq_scr, x_sb[:, cs], ACT.Square,
                                     accum_out=s1[:, 2 + h:3 + h])
            s2 = P1.tile([128, 2], f32)
            nc.vector.tensor_tensor(out=s2[:, 0:1], in0=s1[:, 0:1],
                                    in1=s1[:, 1:2], op=OP.add)
            nc.vector.tensor_tensor(out=s2[:, 1:2], in0=s1[:, 2:3],
                                    in1=s1[:, 3:4], op=OP.add)
            ps_st = PSQ.tile([128, 512], f32, tag="pp", name="ps_st")
            nc.tensor.matmul(ps_st[0:1, 0:16], lhsT=s2[:, 0:1], rhs=sel_sb,
                             start=True, stop=True)
            nc.tensor.matmul(ps_st[0:1, 16:32], lhsT=s2[:, 1:2], rhs=sel_sb,
                             start=True, stop=True)
            stats = P1.tile([1, 32], f32)
            nc.vector.tensor_scalar_mul(stats, ps_st[0:1, 0:32], 1.0 / N_ELEM)
            var = P1.tile([1, 16], f32)
            nc.vector.tensor_mul(var, stats[:, 0:16], stats[:, 0:16])
            nc.vector.tensor_sub(var, stats[:, 16:32], var)
            eps_t = P1.tile([1, 1], f32)
            nc.vector.memset(eps_t, EPS)
            zero_t = P1.tile([1, 1], f32)
            nc.vector.memset(zero_t, 0.0)
            inv = P1.tile([1, 16], f32)
            nc.scalar.activation(inv, var, ACT.Ln, bias=eps_t)
            nc.scalar.activation(inv, inv, ACT.Exp, scale=-0.5, bias=zero_t)
            st_dram = PD.tile([2, 16], f32, name="stb")
            nc.sync.dma_start(out=st_dram[0:1, :], in_=stats[:, 0:16])
            nc.sync.dma_start(out=st_dram[1:2, :], in_=inv)
            mean_p = P1.tile([128, 1], f32)
            inv_p = P1.tile([128, 1], f32)
            for dst, row in ((mean_p, st_dram[0:1, :]),
                             (inv_p, st_dram[1:2, :])):
                src = bass.AP(tensor=row.tensor, offset=row.offset,
                              ap=[[0, T], list(row.ap[-1])])
                nc.gpsimd.dma_start(out=dst[:], in_=src)
            xhat = P1.tile([128, HW], bf16)
            nc.vector.tensor_scalar(out=xhat, in0=x_sb, scalar1=mean_p,
                                    scalar2=inv_p, op0=OP.subtract,
                                    op1=OP.mult)

            # ---------------- q/k projections ----------------
            q_sb = P1.tile([128, HW], bf16)
            k_sb = P1.tile([128, HW], bf16)
            for ti, (dst, wbd) in enumerate(((q_sb, wqbd_sb),
                                             (k_sb, wkbd_sb))):
                for ch in range(4):
                    cs = slice(ch * 1024, (ch + 1) * 1024)
                    ps = PSQ.tile([128, 1024], f32, tag="qk", bufs=3,
                                  name=f"pj{ti}_{ch}")
                    for b in range(2):
                        nc.tensor.matmul(
                            ps[:, b * 512:(b + 1) * 512], lhsT=wbd,
                            rhs=xhat[:, ch * 1024 + b * 512:
                                     ch * 1024 + (b + 1) * 512],
                            start=True, stop=True)
                    if (ch + ti) % 2 == 0:
                        nc.scalar.copy(dst[:, cs], ps)
                    else:
                        nc.vector.tensor_copy(dst[:, cs], ps)
            for rb in (8, 40, 72, 104):
                nc.gpsimd.dma_start(out=q_sb[rb:rb + 1, :], in_=ones_row)

            # ---------------- v -> vt [(jt), l, 9] ----------------
            vt = P1.tile([128, 32 * 18], bf16)
            vt_ap = vt[:]
            ones_vt = bass.AP(tensor=vt_ap.tensor, offset=vt_ap.offset,
                              ap=[list(vt_ap.ap[0]), [18, 32], [9, 2]])
            nc.vector.memset(ones_vt, 1.0)
            for g4 in range(8):
                psv = PSQ.tile([128, 1024], f32, tag="qk", bufs=3,
                               name=f"psv{g4}")
                for c4 in range(4):
                    jc = g4 * 4 + c4
                    nc.tensor.matmul(
                        psv[:, c4 * 18:c4 * 18 + 18],
                        lhsT=xhat[:, jc * 128:(jc + 1) * 128],
                        rhs=wvrhs_sb, start=True, stop=True)
                pv = psv[:]
                src = bass.AP(tensor=pv.tensor, offset=pv.offset + 1,
                              ap=[list(pv.ap[0]), [18, 4], [9, 2], [1, 8]])
                dst = bass.AP(tensor=vt_ap.tensor,
                              offset=vt_ap.offset + g4 * 4 * 18 + 1,
                              ap=[list(vt_ap.ap[0]), [18, 4], [9, 2], [1, 8]])
                nc.vector.tensor_copy(dst, src)

            # ---------------- attention ----------------
            nbias = P1.tile([128, 1], f32)
            nc.vector.memset(nbias, -CBIAS)
            pi = 0  # exp pair counter (engine split)
            for q in range(NQ):
                i0 = q * QW
                av = PSQ.tile([128, 512], f32, tag="av", name=f"av{q}")
                prev = None
                for g2 in range(16):
                    jt0 = g2 * 2
                    # 4 chunks (jt, l) per group; 8 QK matmuls emitted
                    # b-outer so consecutive MMs hit different row
                    # groups and run concurrently on the PE.
                    chunks = [(jt0 + half, l) for half in (0, 1)
                              for l in (0, 1)]
                    qks = [PSQ.tile([128, 1024], f32, tag="qk", bufs=3,
                                    name=f"qk{q}_{g2}_{ci}")
                           for ci in range(4)]
                    # each (chunk, b) = 4 col-group subtile matmuls
                    # (M=32): col tiling gives each stream its own
                    # XBUS, so they fill the array truly in parallel
                    # (full-width M=128 matmuls share one rhs bus).
                    for b in range(2):
                        for ci, (jt, l) in enumerate(chunks):
                            rg = 32 * l + 64 * (jt & 1)
                            for g in range(4):
                                nc.tensor.matmul(
                                    qks[ci][32 * g:32 * g + 32,
                                            b * 512:(b + 1) * 512],
                                    lhsT=k_sb[rg:rg + 9,
                                              jt * 128 + 32 * g:
                                              jt * 128 + 32 * g + 32],
                                    rhs=q_sb[rg:rg + 9,
                                             i0 + b * 512:
                                             i0 + (b + 1) * 512],
                                    start=True, stop=True,
                                    tile_position=(rg, 32 * g))
                    entries = []
                    for ci, (jt, l) in enumerate(chunks):
                        if pi % 2 == 0:
                            ex = PE_.tile([128, 1024], bf16,
                                          tag="exa", bufs=4)
                            nc.scalar.activation(ex, qks[ci], ACT.Exp,
                                                 scale=SCALE, bias=nbias)
                        else:
                            ex = PE_.tile([128, 1024], bf16,
                                          tag="exd", bufs=4)
                            nc.vector.tensor_scalar(
                                out=ex[:].bitcast(i16), in0=qks[ci],
                                scalar1=A_SCH, scalar2=B_SCH,
                                op0=OP.mult, op1=OP.add)
                        pi += 1
                        entries.append((jt, l, ex))
                    if prev is not None:
                        _emit_av(nc, av, vt, prev)
                    prev = entries
                _emit_av(nc, av, vt, prev)

                # ship unnormalized rows + sumexp for this quarter
                s128 = PW.tile([128, 512], bf16, tag="s128")
                nc.scalar.copy(s128, av)
                for l in range(2):
                    for b in range(2):
                        g = 2 * l + b
                        nc.sync.dma_start(
                            out=cc_in[q][l * 9:l * 9 + 9,
                                         b * 512:(b + 1) * 512],
                            in_=s128[32 * g:32 * g + 9, :])
                nc.gpsimd.collective_compute(
                    "AllGather", OP.bypass,
                    replica_groups=[list(range(NCORES))],
                    ins=[cc_in[q].opt()], outs=[cc_out[q].opt()])

                # ---- output phase for this quarter (overlaps next) ----
                rsum = PW.tile([16, QW], bf16, tag="rsum")
                src = bass.AP(tensor=cc_out[q].tensor, offset=0,
                              ap=[[9 * QW, 16], [1, QW]])
                nc.sync.dma_start(out=rsum[:], in_=src)
                rinv = PW.tile([16, QW], f32, tag="rinv")
                nc.vector.reciprocal(rinv, rsum)
                rdram = PD.tile([16, QW], f32, name=f"rd{q}")
                nc.sync.dma_start(out=rdram[:], in_=rinv[:])
                rd_t = rdram[:].tensor
                for hh in range(QW // 512):
                    c0 = i0 + hh * 512
                    rbc = PW.tile([128, 512], f32, tag="rbc")
                    src2 = bass.AP(tensor=rd_t, offset=hh * 512,
                                   ap=[[QW, 16], [0, T], [1, 512]])
                    nc.sync.dma_start(out=rbc[:], in_=src2)
                    acf = PW.tile([128, 512], bf16, tag="acf")
                    src3 = bass.AP(tensor=cc_out[q].tensor,
                                   offset=QW + hh * 512,
                                   ap=[[9 * QW, 16], [QW, T], [1, 512]])
                    nc.sync.dma_start(out=acf[:], in_=src3)
                    att_n = PW.tile([128, 512], bf16, tag="att_n")
                    nc.gpsimd.tensor_tensor(out=att_n, in0=acf, in1=rbc,
                                            op=OP.mult)
                    psp = PSQ.tile([128, 512], f32, tag="pp",
                                   name=f"pp{q}_{hh}")
                    nc.tensor.matmul(psp[:, 0:512], lhsT=wpbd_sb, rhs=att_n,
                                     start=True, stop=True)
                    och = PW.tile([128, 512], f32, tag="och")
                    nc.vector.scalar_tensor_tensor(
                        out=och, in0=psp[:, 0:512], scalar=bpcol_sb,
                        in1=x_sb[:, c0:c0 + 512], op0=OP.add, op1=OP.add)
                    nc.sync.dma_start(out=out[:, c0:c0 + 512], in_=och)

    nc.compile()
    return nc


def _emit_av(nc, av, vt, entries):
    """AV matmuls for one 4-chunk group (4 entries of (jt, l, ex)).
    Emission order interleaves the 4 distinct col groups (cg = 2l + b)
    for PE concurrency; same-cg matmuls accumulate in order."""
    for e0 in (0, 2):
        for b in (0, 1):
            for li in (0, 1):
                jt, l, ex = entries[e0 + li]
                g = 2 * l + b
                nc.tensor.matmul(
                    av[32 * g:32 * g + 9, :],
                    lhsT=vt[:, jt * 18 + l * 9:jt * 18 + l * 9 + 9],
                    rhs=ex[:, b * 512:(b + 1) * 512],
                    start=(jt == 0), stop=(jt == 31),
                    tile_position=(0, 32 * g),
                    skip_group_check=True)


def host_inputs(r, x128, gamma, beta, wq, bq, wk, bk, wv, bv, wp, bp):
    """Per-core host-side input prep (folds gamma/beta/biases)."""
    import ml_dtypes
    bf = ml_dtypes.bfloat16
    wq_e = (wq * gamma[None, :]).astype(np.float32)
    wk_e = (wk * gamma[None, :]).astype(np.float32)
    wv_e = (wv * gamma[None, :]).astype(np.float32)
    bq_e = (bq + wq @ beta).astype(np.float32)
    bv_e = (bv + wv @ beta).astype(np.float32)
    bp_e = (bp + wp @ bv_e).astype(np.float32)
    # bk_e cancels in softmax (adds an i-only term); bq_e enters via the
    # extra k row: k_row8[j] = bq * sum_f k0[f,j].

    wq_bd = np.zeros((128, 128), np.float32)
    wk_bd = np.zeros((128, 128), np.float32)
    wv_rhs = np.zeros((128, 18), np.float32)
    fi = np.arange(T)
    ci = np.arange(C)
    for l in range(2):
        n = 2 * r + l
        for u in range(2):
            base = 32 * l + 64 * u
            wq_bd[fi[:, None] * 16 + ci[None, :], (base + fi)[:, None]] = \
                wq_e[n]
            wk_bd[fi[:, None] * 16 + ci[None, :], (base + fi)[:, None]] = \
                wk_e[n]
            wk_bd[fi[:, None] * 16 + ci[None, :], base + 8] = \
                (bq_e[n] * wk_e[n])[None, :]
        wv_rhs[fi[:, None] * 16 + ci[None, :], (l * 9 + 1 + fi)[:, None]] = \
            wv_e[n]
    wp_bd = np.zeros((128, 128), np.float32)
    bp_col = np.zeros((128, 1), np.float32)
    for f in range(T):
        wp_bd[np.ix_(ci * 8 + f, f * 16 + ci)] = wp.T
        bp_col[f * 16 + ci, 0] = bp_e
    selm = np.zeros((128, 16), np.float32)
    selm[np.arange(128), np.tile(ci, T)] = 1.0
    ones = np.ones((1, HW), np.float32)
    return dict(x=x128, wq_bd=wq_bd.astype(bf), wk_bd=wk_bd.astype(bf),
                wv_rhs=wv_rhs.astype(bf), wp_bd=wp_bd.astype(bf),
                bp_col=bp_col, sel=selm, ones_row=ones.astype(bf))


def make_in_maps(inputs):
    x = np.ascontiguousarray(np.asarray(inputs["x"], np.float32))
    x128 = x.reshape(128, HW)
    args = {k: np.asarray(v, np.float32) for k, v in inputs.items()
            if k != "x"}
    return [host_inputs(r, x128, **args) for r in range(NCORES)]


def run(inputs, trace=False):
    """Returns (out (8,16,64,64) f32, BassKernelResults)."""
    from concourse.bass_utils import run_bass_kernel_spmd
    if "nc" not in _CACHE:
        _CACHE["nc"] = _build_program()
    nc = _CACHE["nc"]
    in_maps = make_in_maps(inputs)
    res = run_bass_kernel_spmd(nc, in_maps, list(range(NCORES)), trace=trace)
    out = np.asarray(res.results[0]["out"], np.float32).reshape(T, C, 64, 64)
    return out, res


def kernel(**inputs):
    out, _ = run(inputs, trace=False)
    return out


# revision 25
# speedup vs baseline: 1.3199x; 1.3199x over previous
"""AttnBlock3D Trainium2 Bass kernel (8 NeuronCores, SPMD), v3.

Per core r: heads n = 2r, 2r+1.  x viewed as [128=(t,c), 4096=hw].

Structure:
  * ONE persistent [128, 3072] f32 PSUM tile = 6 rotating 512-col slots
    (bank-aligned).  Subtile dependency tracking gives the pipeline:
    QK matmul m writes slot m%6 (waits the exp that read it 6 slots
    ago); exp ops read slot PAIRS [128,1024] (mixed heads - exp is
    elementwise); AV consumes ex halves via a slot map.
  * QK matmuls are emitted in pure round-robin over the 4 PE row
    groups (head l replicated at rows 32l and 32l+64, with the bq bias
    folded into k row 8 and q row 8 = ones), so 4 run concurrently.
    AV matmuls (K=128, all rows) for group g-1 are emitted after the
    QK matmuls of group g so they never fence the QK streams.
  * exp split: ACT (exact, bias -c*ln2/128) and DVE (Schraudolph
    int16 bit-trick, bitcast to bf16), alternating slot pairs.
  * i processed in 4 quarters; AV accumulates both heads in one psum
    bank (col group 2l+b); per-quarter bf16 AllGather + output
    projection overlap the following quarters.
"""
import math
import sys

import numpy as np

sys.path.insert(0, "/opt/trn_rl_repo")

T, C, HW, NCORES = 8, 16, 4096, 8
N_ELEM = T * HW
EPS = 1e-5
SCALE = float(T) ** -0.5
NQ = 4            # i-quarters
QW = HW // NQ     # 1024
# Schraudolph: i16 = A*s + B; bf16-bits(i16) ~= exp(SCALE*s - CBIAS)
A_SCH = SCALE * 128.0 / math.log(2.0)
C_SCH = 7.422
B_SCH = 16256.0 - C_SCH
CBIAS = C_SCH * math.log(2.0) / 128.0

_CACHE = {}


def _build_program():
    import concourse.bass as bass
    import concourse.bacc as bacc
    import concourse.tile as tile
    from concourse import mybir

    f32 = mybir.dt.float32
    bf16 = mybir.dt.bfloat16
    i16 = mybir.dt.int16
    AX = mybir.AxisListType
    OP = mybir.AluOpType
    ACT = mybir.ActivationFunctionType

    nc = bacc.Bacc("TRN2", target_bir_lowering=False, debug=False,
                   num_devices=NCORES)
    x = nc.dram_tensor("x", [128, HW], f32, kind="ExternalInput").ap()
    wq_bd = nc.dram_tensor("wq_bd", [128, 128], bf16,
                           kind="ExternalInput").ap()
    wk_bd = nc.dram_tensor("wk_bd", [128, 128], bf16,
                           kind="ExternalInput").ap()
    wv_rhs = nc.dram_tensor("wv_rhs", [128, 18], bf16,
                            kind="ExternalInput").ap()
    wp_bd = nc.dram_tensor("wp_bd", [128, 128], bf16,
                           kind="ExternalInput").ap()
    bp_col = nc.dram_tensor("bp_col", [128, 1], f32, kind="ExternalInput").ap()
    sel = nc.dram_tensor("sel", [128, 16], f32, kind="ExternalInput").ap()
    ones_row = nc.dram_tensor("ones_row", [1, HW], bf16,
                              kind="ExternalInput").ap()
    out = nc.dram_tensor("out", [128, HW], f32, kind="ExternalOutput").ap()

    cc_in = [nc.dram_tensor(f"cc_in{q}", [18, QW], bf16).ap()
             for q in range(NQ)]
    cc_out = [nc.dram_tensor(f"cc_out{q}", [NCORES * 18, QW], bf16,
                             addr_space="Shared").ap() for q in range(NQ)]

    with tile.TileContext(nc) as tc:
        with (
            tc.tile_pool(name="persist", bufs=1) as P1,
            tc.tile_pool(name="work", bufs=4) as PW,
            tc.tile_pool(name="ex", bufs=4) as PE_,
            tc.tile_pool(name="scratch", bufs=1) as PS,
            tc.tile_pool(name="psum", bufs=1, space="PSUM") as PSQ,
            tc.tile_pool(name="dram", bufs=2, space="DRAM") as PD,
        ):
            # psum budget (8 banks): qk 3x[128,1024]=6, av 1, pp 1
            # ---------------- loads ----------------
            x_sb = P1.tile([128, HW], f32)
            nc.sync.dma_start(out=x_sb[:, 0:2048], in_=x[:, 0:2048])
            nc.sync.dma_start(out=x_sb[:, 2048:4096], in_=x[:, 2048:4096])
            wqbd_sb = P1.tile([128, 128], bf16)
            nc.sync.dma_start(out=wqbd_sb, in_=wq_bd)
            wkbd_sb = P1.tile([128, 128], bf16)
            nc.sync.dma_start(out=wkbd_sb, in_=wk_bd)
            wvrhs_sb = P1.tile([128, 18], bf16)
            nc.sync.dma_start(out=wvrhs_sb, in_=wv_rhs)
            wpbd_sb = P1.tile([128, 128], bf16)
            nc.sync.dma_start(out=wpbd_sb, in_=wp_bd)
            bpcol_sb = P1.tile([128, 1], f32)
            nc.sync.dma_start(out=bpcol_sb, in_=bp_col)
            sel_sb = P1.tile([128, 16], f32)
            nc.sync.dma_start(out=sel_sb, in_=sel)

            # PE warm-up: ~3.5us of back-to-back matmuls during the BN
            # stats phase so the HAM clock gate lifts to 2.4 GHz.
            wu = PSQ.tile([128, 512], f32, tag="pp")
            for w in range(32):
                nc.tensor.matmul(wu[:, 0:128], lhsT=wqbd_sb,
                                 rhs=wkbd_sb, start=True, stop=True)

            # ---------------- BN stats ----------------
            s1 = P1.tile([128, 4], f32)
            sq_scr = PS.tile([128, 2048], bf16, tag="sqscr")
            for h in range(2):
                cs = slice(h * 2048, (h + 1) * 2048)
                nc.vector.reduce_sum(out=s1[:, h:h + 1], in_=x_sb[:, cs],
                                     axis=AX.X)
                nc.scalar.activation(sq_scr, x_sb[:, cs], ACT.Square,
                                     accum_out=s1[:, 2 + h:3 + h])
            s2 = P1.tile([128, 2], f32)
            nc.vector.tensor_tensor(out=s2[:, 0:1], in0=s1[:, 0:1],
                                    in1=s1[:, 1:2], op=OP.add)
            nc.vector.tensor_tensor(out=s2[:, 1:2], in0=s1[:, 2:3],
                                    in1=s1[:, 3:4], op=OP.add)
            ps_st = PSQ.tile([128, 512], f32, tag="pp", name="ps_st")
            nc.tensor.matmul(ps_st[0:1, 0:16], lhsT=s2[:, 0:1], rhs=sel_sb,
                             start=True, stop=True)
            nc.tensor.matmul(ps_st[0:1, 16:32], lhsT=s2[:, 1:2], rhs=sel_sb,
                             start=True, stop=True)
            stats = P1.tile([1, 32], f32)
            nc.vector.tensor_scalar_mul(stats, ps_st[0:1, 0:32], 1.0 / N_ELEM)
            var = P1.tile([1, 16], f32)
            nc.vector.tensor_mul(var, stats[:, 0:16], stats[:, 0:16])
            nc.vector.tensor_sub(var, stats[:, 16:32], var)
            eps_t = P1.tile([1, 1], f32)
            nc.vector.memset(eps_t, EPS)
            zero_t = P1.tile([1, 1], f32)
            nc.vector.memset(zero_t, 0.0)
            inv = P1.tile([1, 16], f32)
            nc.scalar.activation(inv, var, ACT.Ln, bias=eps_t)
            nc.scalar.activation(inv, inv, ACT.Exp, scale=-0.5, bias=zero_t)
            st_dram = PD.tile([2, 16], f32, name="stb")
            nc.sync.dma_start(out=st_dram[0:1, :], in_=stats[:, 0:16])
            nc.sync.dma_start(out=st_dram[1:2, :], in_=inv)
            mean_p = P1.tile([128, 1], f32)
            inv_p = P1.tile([128, 1], f32)
            for dst, row in ((mean_p, st_dram[0:1, :]),
                             (inv_p, st_dram[1:2, :])):
                src = bass.AP(tensor=row.tensor, offset=row.offset,
                              ap=[[0, T], list(row.ap[-1])])
                nc.gpsimd.dma_start(out=dst[:], in_=src)
            xhat = P1.tile([128, HW], bf16)
            nc.vector.tensor_scalar(out=xhat, in0=x_sb, scalar1=mean_p,
                                    scalar2=inv_p, op0=OP.subtract,
                                    op1=OP.mult)

            # ---------------- q/k projections ----------------
            q_sb = P1.tile([128, HW], bf16)
            k_sb = P1.tile([128, HW], bf16)
            for ti, (dst, wbd) in enumerate(((q_sb, wqbd_sb),
                                             (k_sb, wkbd_sb))):
                for ch in range(4):
                    cs = slice(ch * 1024, (ch + 1) * 1024)
                    ps = PSQ.tile([128, 1024], f32, tag="qk", bufs=3,
                                  name=f"pj{ti}_{ch}")
                    for b in range(2):
                        nc.tensor.matmul(
                            ps[:, b * 512:(b + 1) * 512], lhsT=wbd,
                            rhs=xhat[:, ch * 1024 + b * 512:
                                     ch * 1024 + (b + 1) * 512],
                            start=True, stop=True)
                    if (ch + ti) % 2 == 0:
                        nc.scalar.copy(dst[:, cs], ps)
                    else:
                        nc.vector.tensor_copy(dst[:, cs], ps)
            for rb in (8, 40, 72, 104):
                nc.gpsimd.dma_start(out=q_sb[rb:rb + 1, :], in_=ones_row)

            # ---------------- v -> vt [(jt), l, 9] ----------------
            vt = P1.tile([128, 32 * 18], bf16)
            vt_ap = vt[:]
            ones_vt = bass.AP(tensor=vt_ap.tensor, offset=vt_ap.offset,
                              ap=[list(vt_ap.ap[0]), [18, 32], [9, 2]])
            nc.vector.memset(ones_vt, 1.0)
            for g4 in range(8):
                psv = PSQ.tile([128, 1024], f32, tag="qk", bufs=3,
                               name=f"psv{g4}")
                for c4 in range(4):
                    jc = g4 * 4 + c4
                    nc.tensor.matmul(
                        psv[:, c4 * 18:c4 * 18 + 18],
                        lhsT=xhat[:, jc * 128:(jc + 1) * 128],
                        rhs=wvrhs_sb, start=True, stop=True)
                pv = psv[:]
                src = bass.AP(tensor=pv.tensor, offset=pv.offset + 1,
                              ap=[list(pv.ap[0]), [18, 4], [9, 2], [1, 8]])
                dst = bass.AP(tensor=vt_ap.tensor,
                              offset=vt_ap.offset + g4 * 4 * 18 + 1,
                              ap=[list(vt_ap.ap[0]), [18, 4], [9, 2], [1, 8]])
                nc.vector.tensor_copy(dst, src)

            # ---------------- attention ----------------
            nbias = P1.tile([128, 1], f32)
            nc.vector.memset(nbias, -CBIAS)
            pi = 0  # exp pair counter (engine split)
            for q in range(NQ):
                i0 = q * QW
                av = PSQ.tile([128, 512], f32, tag="av", name=f"av{q}")
                prev = None
                for g2 in range(16):
                    jt0 = g2 * 2
                    # 4 chunks (jt, l) per group; 8 QK matmuls emitted
                    # b-outer so consecutive MMs hit different row
                    # groups and run concurrently on the PE.
                    chunks = [(jt0 + half, l) for half in (0, 1)
                              for l in (0, 1)]
                    qks = [PSQ.tile([128, 1024], f32, tag="qk", bufs=3,
                                    name=f"qk{q}_{g2}_{ci}")
                           for ci in range(4)]
                    for b in range(2):
                        for ci, (jt, l) in enumerate(chunks):
                            rg = 32 * l + 64 * (jt & 1)
                            nc.tensor.matmul(
                                qks[ci][:, b * 512:(b + 1) * 512],
                                lhsT=k_sb[rg:rg + 9,
                                          jt * 128:(jt + 1) * 128],
                                rhs=q_sb[rg:rg + 9,
                                         i0 + b * 512:i0 + (b + 1) * 512],
                                start=True, stop=True,
                                tile_position=(rg, 0))
                    entries = []
                    for ci, (jt, l) in enumerate(chunks):
                        if pi % 2 == 0:
                            ex = PE_.tile([128, 1024], bf16,
                                          tag="exa", bufs=4)
                            nc.scalar.activation(ex, qks[ci], ACT.Exp,
                                                 scale=SCALE, bias=nbias)
                        else:
                            ex = PE_.tile([128, 1024], bf16,
                                          tag="exd", bufs=4)
                            nc.vector.tensor_scalar(
                                out=ex[:].bitcast(i16), in0=qks[ci],
                                scalar1=A_SCH, scalar2=B_SCH,
                                op0=OP.mult, op1=OP.add)
                        pi += 1
                        entries.append((jt, l, ex))
                    if prev is not None:
                        _emit_av(nc, av, vt, prev)
                    prev = entries
                _emit_av(nc, av, vt, prev)

                # ship unnormalized rows + sumexp for this quarter
                s128 = PW.tile([128, 512], bf16, tag="s128")
                nc.scalar.copy(s128, av)
                for l in range(2):
                    for b in range(2):
                        g = 2 * l + b
                        nc.sync.dma_start(
                            out=cc_in[q][l * 9:l * 9 + 9,
                                         b * 512:(b + 1) * 512],
                            in_=s128[32 * g:32 * g + 9, :])
                nc.gpsimd.collective_compute(
                    "AllGather", OP.bypass,
                    replica_groups=[list(range(NCORES))],
                    ins=[cc_in[q].opt()], outs=[cc_out[q].opt()])

                # ---- output phase for this quarter (overlaps next) ----
                rsum = PW.tile([16, QW], bf16, tag="rsum")
                src = bass.AP(tensor=cc_out[q].tensor, offset=0,
                              ap=[[9 * QW, 16], [1, QW]])
                nc.sync.dma_start(out=rsum[:], in_=src)
                rinv = PW.tile([16, QW], f32, tag="rinv")
                nc.vector.reciprocal(rinv, rsum)
                rdram = PD.tile([16, QW], f32, name=f"rd{q}")
                nc.sync.dma_start(out=rdram[:], in_=rinv[:])
                rd_t = rdram[:].tensor
                for hh in range(QW // 512):
                    c0 = i0 + hh * 512
                    rbc = PW.tile([128, 512], f32, tag="rbc")
                    src2 = bass.AP(tensor=rd_t, offset=hh * 512,
                                   ap=[[QW, 16], [0, T], [1, 512]])
                    nc.sync.dma_start(out=rbc[:], in_=src2)
                    acf = PW.tile([128, 512], bf16, tag="acf")
                    src3 = bass.AP(tensor=cc_out[q].tensor,
                                   offset=QW + hh * 512,
                                   ap=[[9 * QW, 16], [QW, T], [1, 512]])
                    nc.sync.dma_start(out=acf[:], in_=src3)
                    att_n = PW.tile([128, 512], bf16, tag="att_n")
                    nc.gpsimd.tensor_tensor(out=att_n, in0=acf, in1=rbc,
                                            op=OP.mult)
                    psp = PSQ.tile([128, 512], f32, tag="pp",
                                   name=f"pp{q}_{hh}")
                    nc.tensor.matmul(psp[:, 0:512], lhsT=wpbd_sb, rhs=att_n,
                                     start=True, stop=True)
                    och = PW.tile([128, 512], f32, tag="och")
                    nc.vector.scalar_tensor_tensor(
                        out=och, in0=psp[:, 0:512], scalar=bpcol_sb,
                        in1=x_sb[:, c0:c0 + 512], op0=OP.add, op1=OP.add)
                    nc.sync.dma_start(out=out[:, c0:c0 + 512], in_=och)

    nc.compile()
    return nc


def _emit_av(nc, av, vt, entries):
    """AV matmuls for one 4-chunk group (4 entries of (jt, l, ex)).
    Emission order interleaves the 4 distinct col groups (cg = 2l + b)
    for PE concurrency; same-cg matmuls accumulate in order."""
    for e0 in (0, 2):
        for b in (0, 1):
            for li in (0, 1):
                jt, l, ex = entries[e0 + li]
                g = 2 * l + b
                nc.tensor.matmul(
                    av[32 * g:32 * g + 9, :],
                    lhsT=vt[:, jt * 18 + l * 9:jt * 18 + l * 9 + 9],
                    rhs=ex[:, b * 512:(b + 1) * 512],
                    start=(jt == 0), stop=(jt == 31),
                    tile_position=(0, 32 * g),
                    skip_group_check=True)


def host_inputs(r, x128, gamma, beta, wq, bq, wk, bk, wv, bv, wp, bp):
    """Per-core host-side input prep (folds gamma/beta/biases)."""
    import ml_dtypes
    bf = ml_dtypes.bfloat16
    wq_e = (wq * gamma[None, :]).astype(np.float32)
    wk_e = (wk * gamma[None, :]).astype(np.float32)
    wv_e = (wv * gamma[None, :]).astype(np.float32)
    bq_e = (bq + wq @ beta).astype(np.float32)
    bv_e = (bv + wv @ beta).astype(np.float32)
    bp_e = (bp + wp @ bv_e).astype(np.float32)
    # bk_e cancels in softmax (adds an i-only term); bq_e enters via the
    # extra k row: k_row8[j] = bq * sum_f k0[f,j].

    wq_bd = np.zeros((128, 128), np.float32)
    wk_bd = np.zeros((128, 128), np.float32)
    wv_rhs = np.zeros((128, 18), np.float32)
    fi = np.arange(T)
    ci = np.arange(C)
    for l in range(2):
        n = 2 * r + l
        for u in range(2):
            base = 32 * l + 64 * u
            wq_bd[fi[:, None] * 16 + ci[None, :], (base + fi)[:, None]] = \
                wq_e[n]
            wk_bd[fi[:, None] * 16 + ci[None, :], (base + fi)[:, None]] = \
                wk_e[n]
            wk_bd[fi[:, None] * 16 + ci[None, :], base + 8] = \
                (bq_e[n] * wk_e[n])[None, :]
        wv_rhs[fi[:, None] * 16 + ci[None, :], (l * 9 + 1 + fi)[:, None]] = \
            wv_e[n]
    wp_bd = np.zeros((128, 128), np.float32)
    bp_col = np.zeros((128, 1), np.float32)
    for f in range(T):
        wp_bd[np.ix_(ci * 8 + f, f * 16 + ci)] = wp.T
        bp_col[f * 16 + ci, 0] = bp_e
    selm = np.zeros((128, 16), np.float32)
    selm[np.arange(128), np.tile(ci, T)] = 1.0
    ones = np.ones((1, HW), np.float32)
    return dict(x=x128, wq_bd=wq_bd.astype(bf), wk_bd=wk_bd.astype(bf),
                wv_rhs=wv_rhs.astype(bf), wp_bd=wp_bd.astype(bf),
                bp_col=bp_col, sel=selm, ones_row=ones.astype(bf))


def make_in_maps(inputs):
    x = np.ascontiguousarray(np.asarray(inputs["x"], np.float32))
    x128 = x.reshape(128, HW)
    args = {k: np.asarray(v, np.float32) for k, v in inputs.items()
            if k != "x"}
    return [host_inputs(r, x128, **args) for r in range(NCORES)]


def run(inputs, trace=False):
    """Returns (out (8,16,64,64) f32, BassKernelResults)."""
    from concourse.bass_utils import run_bass_kernel_spmd
    if "nc" not in _CACHE:
        _CACHE["nc"] = _build_program()
    nc = _CACHE["nc"]
    in_maps = make_in_maps(inputs)
    res = run_bass_kernel_spmd(nc, in_maps, list(range(NCORES)), trace=trace)
    out = np.asarray(res.results[0]["out"], np.float32).reshape(T, C, 64, 64)
    return out, res


def kernel(**inputs):
    out, _ = run(inputs, trace=False)
    return out
